# revision 25
# baseline (speedup 1.0000x reference)
"""Trainium2 kernel for nn_ImageStitchingLayer: 2x2 stitching NCC loss.

Math: for z_weights in [0,1), the reference's z-interpolation is a 2-tap blend
s[k] = (1-f)*x[k] + f*x[k-1] (zero-padded to Z+1 planes).  Every sum in the
NCC loss then decomposes into z-lag Gram statistics of the raw overlap slabs:

    sum(s)        = S                    (independent of f)
    sum(s^2)      = ((1-f)^2 + f^2) A + 2 f (1-f) B
    sum(s1 s2)    = ((1-f1)(1-f2) + f1 f2) C0 + (1-f1) f2 Cp + f1 (1-f2) Cm

with S = sum(x), A = sum(x^2), B = sum(x[z] x[z-1]), C0/Cp/Cm the lag-0/+-1
cross sums.  All of these are entries of the z-by-z Gram matrix of the two
slabs, contracted over the 16k hw positions per core.  The device computes
the Gram matrices on the tensor engine (fp8-e4m3 inputs, fp32 PSUM
accumulation); the host combines them in float64.

Sharding: 4 adjacent pairs x 2 half-slabs = 8 cores.  Each core receives only
its two overlap slab halves (48 x 64 x 256 x 2 each), packed host-side as
[128 partitions(hw) x (chunk, ch, [x1 z | x2 z | 1])] fp8.

Schedule: staged DMA pieces (small first so the PE starts early), warmup
matmuls to lift the PE HAM clock gate to 2.4 GHz before real data lands,
channel-0 results copied out (ACT, table pre-loaded) and DMA'd while
channel-1's matmuls still run; only channel-1's copy + DMA + receipt remain
on the tail.  Hard-won HW facts: keep out-DMAs 128-partition column slices
(97-partition or per-channel-contiguous layouts measured 2-5x slower); keep
all DMAs on the sync HWDGE ring (ACT-ring out-DMAs serialized); lhsT bases
4B-aligned via ZSTRIDE=100 (97-byte strides paced the PE at LDWEIGHTS rate).
"""

import numpy as np
import ml_dtypes

Z, H, W = 48, 512, 512
OH = 64
NCH = 2
PAIRS = [(1, 0, "h"), (2, 0, "v"), (3, 1, "v"), (3, 2, "h")]
NCORES = 8

ZCOLS = 2 * Z + 1  # 97: x1 z-planes | x2 z-planes | ones column
ZSTRIDE = 100  # 97 padded to a 4B multiple so every lhsT base is 32b-aligned (FWL)
NCHUNK = 128  # hw chunks of 128 partitions each (16384 hw positions / core)
CHUNK_COLS = NCH * ZSTRIDE  # 200
PIECE_PAD = 32  # so a 128-wide (FWL-eligible) lhsT never overruns its piece
PIECE_SIZES = [8, 8, 16, 16, 16, 16, 16, 16, 16]  # chunks per DMA piece
assert sum(PIECE_SIZES) == NCHUNK
PIECE_W = [n * CHUNK_COLS + PIECE_PAD for n in PIECE_SIZES]
PIECE_OFF = [sum(PIECE_W[:j]) for j in range(len(PIECE_SIZES))]
TOTAL_W = sum(PIECE_W)  # free-dim bytes per partition (fp8)
OUT_COLS = NCH * ZCOLS  # 194
N_WARMUP = 44  # ~2.3us of cold N=64 matmuls to trip the HAM clock gate

_CACHE = {}

LAST_RESULT = None  # BassKernelResults of the most recent device run (for test harness)


def _build_bass():
    """Raw bass (no TileContext): this container's walrus rejects >3 sem waits on
    one instruction, which Tile's kernel-tail drain always exceeds.  Manual sync
    keeps every instruction at <=1 wait."""
    import concourse.bass as bass
    from concourse import mybir

    nc = bass.Bass()
    in_dt = mybir.dt.float8e4
    x = nc.dram_tensor("x", [128, TOTAL_W], in_dt, kind="ExternalInput")
    # NOTE: out stays 128 partitions x column-slices -- sub-128-partition DMAs
    # and per-channel contiguous blocks both measured 2-5x SLOWER (HW probe)
    out = nc.dram_tensor("out", [128, OUT_COLS], mybir.dt.float32, kind="ExternalOutput")

    npieces = len(PIECE_SIZES)

    with (
        nc.sbuf_tensor([128, TOTAL_W], in_dt) as data,
        nc.sbuf_tensor([128, OUT_COLS], mybir.dt.float32) as out_t,
        nc.sbuf_tensor([128, 192], in_dt) as junk,
        nc.sbuf_tensor([128, 8], mybir.dt.float32) as junk_f32,
        nc.psum_tensor([128, ZCOLS], mybir.dt.float32) as ps0,
        nc.psum_tensor([128, ZCOLS], mybir.dt.float32) as ps1,
        nc.psum_tensor([128, 64], mybir.dt.float32) as warm_ps,
        nc.semaphore() as dma_sem,
        nc.semaphore() as pe_sem,
        nc.semaphore() as cp0_sem,
        nc.semaphore() as cp1_sem,
        nc.Block() as block,
    ):
        psums = [ps0, ps1]

        @block.sync
        def _(sync):
            for j in range(npieces):
                sync.dma_start(
                    data[:, PIECE_OFF[j] : PIECE_OFF[j] + PIECE_W[j]],
                    x[:, PIECE_OFF[j] : PIECE_OFF[j] + PIECE_W[j]],
                ).then_inc(dma_sem, 16)
            sync.wait_ge(cp0_sem, 1)
            sync.dma_start(out[:, 0:ZCOLS], out_t[:, 0:ZCOLS]).then_inc(dma_sem, 16)
            sync.wait_ge(cp1_sem, 1)
            sync.dma_start(
                out[:, ZCOLS : 2 * ZCOLS], out_t[:, ZCOLS : 2 * ZCOLS]
            ).then_inc(dma_sem, 16)
            sync.wait_ge(dma_sem, (npieces + 2) * 16)

        @block.tensor
        def _(tensor):
            # Warmup: PE would otherwise idle here waiting for DMA; cold junk
            # matmuls make the HAM activity window read busy so the real
            # matmuls run at 2.4 GHz instead of 1.2.
            for _ in range(N_WARMUP):
                tensor.matmul(
                    warm_ps[:, :], junk[:, 0:128], junk[:, 128:192], start=True, stop=True
                )
            done = [0, 0]
            for j in range(npieces):
                tensor.wait_ge(dma_sem, (j + 1) * 16)
                for c in range(NCH):
                    for k in range(PIECE_SIZES[j]):
                        base = PIECE_OFF[j] + (k * NCH + c) * ZSTRIDE
                        mm = tensor.matmul(
                            psums[c][:, :],
                            data[:, base : base + 128],  # stationary (tail cols junk)
                            data[:, base : base + ZCOLS],  # moving [128hw, 97]
                            start=(done[c] == 0),
                            stop=(done[c] == NCHUNK - 1),
                        )
                        done[c] += 1
                        if done[c] == NCHUNK:
                            # channel complete: release its copy engine while the
                            # other channel's matmuls continue
                            mm.then_inc(pe_sem, 1)

        @block.scalar
        def _(scalar):
            # Dummy copy so the ACT engine's lazy activation-table load (~1.3us)
            # happens under the DMA phase, not on the critical tail.
            scalar.copy(junk_f32[:, 0:4], junk_f32[:, 4:8])
            # pe_sem hits 1 when channel 0's accumulation is complete (channel 1's
            # last matmuls still running) -- copy + DMA channel 0 under them.
            scalar.wait_ge(pe_sem, 1)
            scalar.copy(out_t[:, 0:ZCOLS], ps0[:, :]).then_inc(cp0_sem, 1)
            scalar.wait_ge(pe_sem, 2)
            scalar.copy(out_t[:, ZCOLS : 2 * ZCOLS], ps1[:, :]).then_inc(cp1_sem, 1)

    return nc


def _pack_core(x1, x2):
    """x1, x2: [Z, OH, 256, NCH] float32 -> [128, TOTAL_W] fp8-e4m3."""

    def r(x):  # -> [chunk, p, ch, z]
        # [Z, 64, 256, c] -> [64, 256, c, Z] -> [hw, c, Z] -> [chunk, p, c, Z]
        return np.ascontiguousarray(x.transpose(1, 2, 3, 0)).reshape(NCHUNK, 128, NCH, Z)

    dt = ml_dtypes.float8_e4m3
    x1r = r(x1)
    x2r = r(x2)
    D = np.zeros((128, NCHUNK, NCH, ZSTRIDE), dtype=dt)  # [p, k, c, q]
    D[:, :, :, 0:Z] = x1r.transpose(1, 0, 2, 3)
    D[:, :, :, Z : 2 * Z] = x2r.transpose(1, 0, 2, 3)
    D[:, :, :, 2 * Z] = 1.0
    X = np.zeros((128, TOTAL_W), dtype=dt)
    a = 0
    for j, n in enumerate(PIECE_SIZES):
        X[:, PIECE_OFF[j] : PIECE_OFF[j] + n * CHUNK_COLS] = D[:, a : a + n].reshape(
            128, n * CHUNK_COLS
        )
        a += n
    return X


def _slabs(stacks):
    """Yield (x1_half, x2_half) float32 views/copies per core, canonical [Z,64,512,2] split in two."""
    out = []
    for i, j, ori in PAIRS:
        if ori == "v":
            a = stacks[i][:, 0:OH, :, :]
            b = stacks[j][:, H - OH : H, :, :]
        else:
            a = stacks[i][:, :, 0:OH, :].transpose(0, 2, 1, 3)
            b = stacks[j][:, :, W - OH : W, :].transpose(0, 2, 1, 3)
        for half in range(2):
            sl = slice(half * 256, (half + 1) * 256)
            out.append((a[:, :, sl, :], b[:, :, sl, :]))
    return out


def _run_device(in_maps, trace=False):
    global LAST_RESULT
    from concourse import bass_utils

    if "nc" not in _CACHE:
        _CACHE["nc"] = _build_bass()
    for _attempt in range(3):
        res = bass_utils.run_bass_kernel_spmd(
            _CACHE["nc"], in_maps, core_ids=list(range(NCORES)), trace=trace
        )
        LAST_RESULT = res
        ok = all(np.isfinite(r["out"]).all() and np.abs(r["out"]).sum() > 0 for r in res.results)
        if ok:
            break
    return res.results


def kernel(stacks, z_weights):
    stacks = np.asarray(stacks, dtype=np.float32)
    zw = np.asarray(z_weights, dtype=np.float64)

    in_maps = [{"x": _pack_core(x1, x2)} for (x1, x2) in _slabs(stacks)]
    results = _run_device(in_maps)

    N = (Z + 1) * OH * W
    loss = 0.0
    for p_idx, (i, j, _ori) in enumerate(PAIRS):
        f1, f2 = zw[i], zw[j]
        O = results[2 * p_idx]["out"].astype(np.float64) + results[2 * p_idx + 1][
            "out"
        ].astype(np.float64)
        for c in range(NCH):
            M = O[:, c * ZCOLS : (c + 1) * ZCOLS]
            G11 = M[0:Z, 0:Z]
            G12 = M[0:Z, Z : 2 * Z]
            G22 = M[Z : 2 * Z, Z : 2 * Z]
            S1 = M[0:Z, 2 * Z].sum()
            S2 = M[Z : 2 * Z, 2 * Z].sum()
            A1 = np.trace(G11)
            B1 = np.trace(G11, offset=-1)
            A2 = np.trace(G22)
            B2 = np.trace(G22, offset=-1)
            C0 = np.trace(G12)
            Cp = np.trace(G12, offset=-1)  # sum_z x1[z] x2[z-1]
            Cm = np.trace(G12, offset=1)  # sum_z x1[z-1] x2[z]
            ss1 = ((1 - f1) ** 2 + f1**2) * A1 + 2 * f1 * (1 - f1) * B1
            ss2 = ((1 - f2) ** 2 + f2**2) * A2 + 2 * f2 * (1 - f2) * B2
            s12 = (
                ((1 - f1) * (1 - f2) + f1 * f2) * C0
                + (1 - f1) * f2 * Cp
                + f1 * (1 - f2) * Cm
            )
            m11 = ss1 - S1 * S1 / N
            m22 = ss2 - S2 * S2 / N
            m12 = s12 - S1 * S2 / N
            loss += m12**2 + m11 * m22
    return np.array(loss, dtype=np.float32)
